# revision 16
# baseline (speedup 1.0000x reference)
"""Multi-head attention + residual LayerNorm Trainium2 kernel (8 NeuronCores).

Sharding: core c -> batch b = c//4, head group hg = c%4 (heads 4*hg .. 4*hg+3).
Tensor-parallel over heads for W_Q/W_K/W_V (column) and W_fc (row), with a
4-way ReduceScatter per batch group before the residual LayerNorm.

Outputs per core: attn probabilities for its 4 heads ([4, S, S], [q, k] layout)
and its ReduceScatter shard of the LayerNorm output ([4, 128, D]).
"""

import sys

sys.path.insert(0, "/opt/trn_rl_repo")

import numpy as np

import concourse.bass as bass
import concourse.mybir as mybir
from concourse import bacc
from concourse.tile import TileContext
from concourse.bass_utils import run_bass_kernel_spmd
from concourse.masks import make_identity

B = 2
S = 2048
D = 1024
H_TOT = 16
H_LOC = 4  # heads per core
DK = 64
EPS = 1e-5
N_CORES = 8

F32 = mybir.dt.float32
F32R = mybir.dt.float32r
U8 = mybir.dt.uint8

QT_CH = 2       # 256 local out-features in 2 chunks of 128
D_CH = D // 128   # 8 dmodel chunks
TOK_CH = S // 512  # 4 token chunks of 512
KT_N = S // 128   # 16 key tiles of 128
QT_N = S // 128   # 16 query tiles of 128


def build_nc():
    nc = bacc.Bacc("TRN2", target_bir_lowering=False, debug=False, num_devices=N_CORES)

    xt_q = nc.declare_dram_parameter("xt_q", [D, S], F32R, isOutput=False)
    xt_k = nc.declare_dram_parameter("xt_k", [D, S], F32R, isOutput=False)
    xt_v = nc.declare_dram_parameter("xt_v", [D, S], F32R, isOutput=False)
    wq_t = nc.declare_dram_parameter("wq_t", [D, 256], F32R, isOutput=False)
    wk_t = nc.declare_dram_parameter("wk_t", [D, 256], F32R, isOutput=False)
    wv_t = nc.declare_dram_parameter("wv_t", [D, 256], F32R, isOutput=False)
    wfc_t = nc.declare_dram_parameter("wfc_t", [256, D], F32R, isOutput=False)
    notmask = nc.declare_dram_parameter("notmask", [S, S], U8, isOutput=False)
    notmask_t = nc.declare_dram_parameter("notmask_t", [S, S], U8, isOutput=False)
    resid = nc.declare_dram_parameter("resid", [TOK_CH, 128, D], F32, isOutput=False)

    attn_out = nc.declare_dram_parameter("attn_out", [H_LOC, S, S], F32, isOutput=True)
    ln_out = nc.declare_dram_parameter("ln_out", [TOK_CH, 128, D], F32, isOutput=True)

    partial = nc.dram_tensor("partial", [S, D], F32)
    rs_out = nc.dram_tensor("rs_out", [TOK_CH, 128, D], F32)
    groups = [[0, 1, 2, 3], [4, 5, 6, 7]]

    with TileContext(nc) as tc:
        with (
            tc.tile_pool(name="persist", bufs=1) as pp,
            tc.tile_pool(name="stream", bufs=3) as sp,
            tc.tile_pool(name="wpool", bufs=8) as wp,
            tc.tile_pool(name="small", bufs=2) as mp,
            tc.tile_pool(name="psum", bufs=8, space="PSUM") as ps,
        ):
            # ---------------- constants ----------------
            ident = pp.tile([128, 128], F32, tag="ident")
            make_identity(nc, ident)
            ones4 = pp.tile([128, 4], F32, tag="ones4")
            nc.vector.memset(ones4, 1.0)
            ones_row = pp.tile([1, DK], F32, tag="onesr")
            nc.vector.memset(ones_row, 1.0)
            eps_t = pp.tile([128, 1], F32, tag="eps")
            nc.vector.memset(eps_t, EPS)

            # persistent activation tensors
            qt_sb = [pp.tile([128, S], F32R, tag=f"qt{i}", name=f"qt{i}") for i in range(QT_CH)]
            kt_sb = [pp.tile([128, S], F32R, tag=f"kt{i}", name=f"kt{i}") for i in range(QT_CH)]
            vtt_sb = [pp.tile([128, S], F32, tag=f"vtt{i}", name=f"vtt{i}") for i in range(QT_CH)]
            ctxt_sb = [pp.tile([128, S], F32R, tag=f"ctxt{i}", name=f"ctxt{i}") for i in range(QT_CH)]
            vt_sb = [pp.tile([128, H_LOC, DK + 1], F32R, tag=f"vt{i}", name=f"vt{i}") for i in range(KT_N)]
            wfc_sb = [pp.tile([128, D], F32R, tag=f"wfc{i}", name=f"wfc{i}") for i in range(QT_CH)]
            sums_col = pp.tile([128, QT_N, H_LOC], F32, tag="sumscol")
            lnneg_col = pp.tile([128, QT_N, H_LOC], F32, tag="lncol")

            # ---------------- projections ----------------
            def project_to_T(xt_dram, w_dram, dst):
                """dst[oc][:, :] = (W_slice @ X^T)[oc*128:(oc+1)*128, :]  (f32r)."""
                wts = [wp.tile([128, 256], F32R, tag="w", name="wt") for _ in range(D_CH)]
                for dc in range(D_CH):
                    nc.sync.dma_start(out=wts[dc], in_=w_dram[dc * 128:(dc + 1) * 128, :])
                accs = [ps.tile([128, 512], F32, tag="bank", name="acc") for _ in range(8)]
                for dc in range(D_CH):
                    xt = sp.tile([128, S], F32R, tag="xt", bufs=2)
                    nc.sync.dma_start(out=xt, in_=xt_dram[dc * 128:(dc + 1) * 128, :])
                    for oc in range(QT_CH):
                        for t in range(TOK_CH):
                            nc.tensor.matmul(
                                accs[oc * TOK_CH + t][:, :],
                                wts[dc][:, oc * 128:(oc + 1) * 128],
                                xt[:, t * 512:(t + 1) * 512],
                                start=(dc == 0),
                                stop=(dc == D_CH - 1),
                            )
                for oc in range(QT_CH):
                    for t in range(TOK_CH):
                        nc.any.tensor_copy(
                            out=dst[oc][:, t * 512:(t + 1) * 512],
                            in_=accs[oc * TOK_CH + t][:, :],
                        )

            project_to_T(xt_q, wq_t, qt_sb)
            project_to_T(xt_k, wk_t, kt_sb)
            project_to_T(xt_v, wv_t, vtt_sb)

            # load W_fc^T chunks
            for oc in range(QT_CH):
                nc.sync.dma_start(out=wfc_sb[oc], in_=wfc_t[oc * 128:(oc + 1) * 128, :])

            # V^T -> V tiles [tok 128, (h, 65)] with trailing ones column
            for kt in range(KT_N):
                for oc in range(QT_CH):
                    vtp = ps.tile([128, 128], F32, tag="bank")
                    nc.tensor.transpose(
                        vtp[:, :], vtt_sb[oc][:, kt * 128:(kt + 1) * 128], ident[:, :]
                    )
                    nc.any.tensor_copy(
                        out=vt_sb[kt][:, oc * 2:(oc + 1) * 2, 0:DK],
                        in_=vtp.rearrange("p (h x) -> p h x", h=2),
                    )
                nc.any.tensor_copy(
                    out=vt_sb[kt][:, :, DK:DK + 1].rearrange("p h x -> p (h x)"),
                    in_=ones4[:, :],
                )

            # ---------------- phase O2: P^T, context, sums ----------------
            for t in range(TOK_CH):  # q chunk of 512
                ctx_acc = [ps.tile([DK + 1, 512], F32, tag="bank", name="ctxacc") for _ in range(H_LOC)]
                for kt in range(KT_N):
                    nm = sp.tile([128, 512], U8, tag="nm2")
                    nc.sync.dma_start(
                        out=nm,
                        in_=notmask_t[kt * 128:(kt + 1) * 128, t * 512:(t + 1) * 512],
                    )
                    for h in range(H_LOC):
                        hc, ho = h // 2, (h % 2) * DK
                        sps = ps.tile([128, 512], F32, tag="bank")
                        nc.tensor.matmul(
                            sps[:, :],
                            kt_sb[hc][ho:ho + DK, kt * 128:(kt + 1) * 128],
                            qt_sb[hc][ho:ho + DK, t * 512:(t + 1) * 512],
                            start=True,
                            stop=True,
                        )
                        pt = sp.tile([128, 512], F32R, tag="pt")
                        nc.scalar.activation(
                            pt[:, :], sps[:, :], mybir.ActivationFunctionType.Exp,
                            scale=0.125,
                        )
                        nc.vector.tensor_tensor(
                            out=pt[:, :], in0=pt[:, :], in1=nm[:, :],
                            op=mybir.AluOpType.mult,
                        )
                        nc.tensor.matmul(
                            ctx_acc[h][:, :],
                            vt_sb[kt][:, h, :],
                            pt[:, :],
                            start=(kt == 0),
                            stop=(kt == KT_N - 1),
                        )
                for h in range(H_LOC):
                    hc, ho = h // 2, (h % 2) * DK
                    sums0 = sp.tile([1, 512], F32, tag="sums0")
                    nc.any.tensor_copy(out=sums0[:, :], in_=ctx_acc[h][DK:DK + 1, :])
                    # sums -> column layout [q 128, qt, h] via tiny transposes
                    scps = ps.tile([128, 4], F32, tag="bank")
                    for j in range(4):
                        nc.tensor.transpose(
                            scps[:, j:j + 1], sums0[0:1, j * 128:(j + 1) * 128],
                            ident[0:1, 0:1],
                        )
                    nc.any.tensor_copy(
                        out=sums_col[:, t * 4:(t + 1) * 4, h], in_=scps[:, :]
                    )
                    # broadcast sums across 64 partitions via K=1 matmul,
                    # then fast reciprocal on the broadcast
                    bps = ps.tile([DK, 512], F32, tag="bank")
                    nc.tensor.matmul(
                        bps[:, :], ones_row[:, :], sums0[:, :],
                        start=True, stop=True,
                    )
                    rec = sp.tile([DK, 512], F32, tag="rec")
                    recs = sp.tile([DK, 512], F32, tag="recs")
                    nc.vector.reciprocal_approx_accurate(
                        out=rec[:, :], in_=bps[:, :], scratch=recs[:, :]
                    )
                    nc.vector.tensor_tensor(
                        out=ctxt_sb[hc][ho:ho + DK, t * 512:(t + 1) * 512],
                        in0=ctx_acc[h][0:DK, :],
                        in1=rec[:, :],
                        op=mybir.AluOpType.mult,
                    )

                # ----- fc for this q chunk + chunked reduce-scatter -----
                for q4 in range(4):
                    q0 = t * 512 + q4 * 128
                    po = sp.tile([128, D], F32, tag="po")
                    for ot in range(QT_CH):
                        fps = ps.tile([128, 512], F32, tag="bank")
                        for hc in range(QT_CH):
                            nc.tensor.matmul(
                                fps[:, :],
                                ctxt_sb[hc][:, q0:q0 + 128],
                                wfc_sb[hc][:, ot * 512:(ot + 1) * 512],
                                start=(hc == 0),
                                stop=(hc == QT_CH - 1),
                            )
                        nc.any.tensor_copy(out=po[:, ot * 512:(ot + 1) * 512], in_=fps[:, :])
                    nc.sync.dma_start(out=partial[q0:q0 + 128, :], in_=po[:, :])
                nc.gpsimd.collective_compute(
                    "ReduceScatter",
                    mybir.AluOpType.add,
                    replica_groups=groups,
                    ins=[partial[t * 512:(t + 1) * 512, :]],
                    outs=[rs_out[t, :, :]],
                )

            # ---------------- lnsum columns for O1 bias ----------------
            nc.scalar.activation(
                lnneg_col.rearrange("p a b -> p (a b)"),
                sums_col.rearrange("p a b -> p (a b)"),
                mybir.ActivationFunctionType.Ln,
            )
            nc.vector.tensor_scalar_mul(
                lnneg_col.rearrange("p a b -> p (a b)"),
                lnneg_col.rearrange("p a b -> p (a b)"),
                -1.0,
            )

            # ---------------- phase O1: attention probabilities ----------------
            for qt in range(QT_N):
                nm1 = mp.tile([128, S], U8, tag="nm1")
                nc.sync.dma_start(out=nm1, in_=notmask[qt * 128:(qt + 1) * 128, :])
                for h in range(H_LOC):
                    hc, ho = h // 2, (h % 2) * DK
                    at = sp.tile([128, S], F32, tag="attn", bufs=2)
                    for kc in range(TOK_CH):
                        sps = ps.tile([128, 512], F32, tag="bank")
                        nc.tensor.matmul(
                            sps[:, :],
                            qt_sb[hc][ho:ho + DK, qt * 128:(qt + 1) * 128],
                            kt_sb[hc][ho:ho + DK, kc * 512:(kc + 1) * 512],
                            start=True,
                            stop=True,
                        )
                        nc.scalar.activation(
                            at[:, kc * 512:(kc + 1) * 512], sps[:, :],
                            mybir.ActivationFunctionType.Exp,
                            scale=0.125,
                            bias=lnneg_col[:, qt, h:h + 1],
                        )
                    nc.vector.tensor_tensor(
                        out=at[:, :], in0=at[:, :], in1=nm1[:, :],
                        op=mybir.AluOpType.mult,
                    )
                    nc.sync.dma_start(
                        out=attn_out[h, qt * 128:(qt + 1) * 128, :], in_=at[:, :]
                    )

            # ---------------- residual + LayerNorm on RS shards ----------------
            for t in range(TOK_CH):
                xr = mp.tile([128, D], F32, tag="lnx", bufs=1)
                nc.sync.dma_start(out=xr, in_=rs_out[t, :, :])
                rt = mp.tile([128, D], F32, tag="lnr", bufs=1)
                nc.sync.dma_start(out=rt, in_=resid[t, :, :])
                nc.vector.tensor_tensor(
                    out=xr[:, :], in0=xr[:, :], in1=rt[:, :], op=mybir.AluOpType.add
                )
                stats = mp.tile([128, 2, 6], F32, tag="lnstats", bufs=1)
                xg = xr.rearrange("p (g d) -> p g d", g=2)
                for g in range(2):
                    nc.vector.bn_stats(out=stats[:, g, :], in_=xg[:, g, :])
                mv = mp.tile([128, 2], F32, tag="lnmv", bufs=1)
                nc.vector.bn_aggr(out=mv[:, :], in_=stats[:, :, :])
                std = mp.tile([128, 1], F32, tag="lnstd", bufs=1)
                nc.scalar.activation(
                    out=std[:, :], in_=mv[:, 1:2],
                    func=mybir.ActivationFunctionType.Sqrt,
                    bias=eps_t[:, 0:1],
                )
                rstd = mp.tile([128, 1], F32, tag="lnrstd", bufs=1)
                nc.vector.reciprocal(out=rstd[:, :], in_=std[:, :])
                ot = mp.tile([128, D], F32, tag="lno", bufs=1)
                nc.vector.tensor_scalar(
                    out=ot[:, :], in0=xr[:, :],
                    scalar1=mv[:, 0:1], scalar2=rstd[:, 0:1],
                    op0=mybir.AluOpType.subtract, op1=mybir.AluOpType.mult,
                )
                nc.sync.dma_start(out=ln_out[t, :, :], in_=ot[:, :])

    nc.compile()
    return nc


_NC = None


def _get_nc():
    global _NC
    if _NC is None:
        _NC = build_nc()
    return _NC


def prepare_in_maps(input_Q, input_K, input_V, attn_mask, W_Q, W_K, W_V, W_fc):
    xts = {}
    nms = {}
    for b in range(B):
        xts[b] = (
            np.ascontiguousarray(input_Q[b].T),
            np.ascontiguousarray(input_K[b].T),
            np.ascontiguousarray(input_V[b].T),
        )
        nm = (~attn_mask[b]).astype(np.uint8)
        nms[b] = (nm, np.ascontiguousarray(nm.T))

    in_maps = []
    for c in range(N_CORES):
        b, hg = c // H_LOC, c % H_LOC
        cols = slice(hg * 256, (hg + 1) * 256)
        res = input_Q[b].reshape(TOK_CH, 4, 128, D)[:, hg, :, :]
        in_maps.append({
            "xt_q": xts[b][0],
            "xt_k": xts[b][1],
            "xt_v": xts[b][2],
            "wq_t": np.ascontiguousarray(W_Q[cols, :].T),
            "wk_t": np.ascontiguousarray(W_K[cols, :].T),
            "wv_t": np.ascontiguousarray(W_V[cols, :].T),
            "wfc_t": np.ascontiguousarray(W_fc[:, cols].T),
            "notmask": nms[b][0],
            "notmask_t": nms[b][1],
            "resid": np.ascontiguousarray(res),
        })
    return in_maps


def assemble(results):
    attn = np.empty((B, H_TOT, S, S), dtype=np.float32)
    ln = np.empty((B, S, D), dtype=np.float32)
    for c in range(N_CORES):
        b, hg = c // H_LOC, c % H_LOC
        r = results[c]
        attn[b, hg * H_LOC:(hg + 1) * H_LOC] = r["attn_out"]
        ln[b].reshape(TOK_CH, 4, 128, D)[:, hg, :, :] = r["ln_out"]
    return ln, attn


def kernel(input_Q, input_K, input_V, attn_mask, W_Q, W_K, W_V, W_fc):
    input_Q = np.asarray(input_Q, dtype=np.float32)
    input_K = np.asarray(input_K, dtype=np.float32)
    input_V = np.asarray(input_V, dtype=np.float32)
    attn_mask = np.asarray(attn_mask)
    W_Q = np.asarray(W_Q, dtype=np.float32)
    W_K = np.asarray(W_K, dtype=np.float32)
    W_V = np.asarray(W_V, dtype=np.float32)
    W_fc = np.asarray(W_fc, dtype=np.float32)

    nc = _get_nc()
    in_maps = prepare_in_maps(input_Q, input_K, input_V, attn_mask,
                              W_Q, W_K, W_V, W_fc)
    try:
        res = run_bass_kernel_spmd(nc, in_maps, list(range(N_CORES)))
    except Exception:
        # one retry: a previously wedged core often recovers on re-attach
        res = run_bass_kernel_spmd(nc, in_maps, list(range(N_CORES)))
    return assemble(res.results)
